# revision 7
# baseline (speedup 1.0000x reference)
"""Trainium2 Bass kernel for AttentionNet:
out[b,h,i,j] = relu(sum_d w2[d] * Xf[b,h,i,d] * Yf[b,h,j,d] + b2)
where Xf = X @ W1.T + b1, Yf = Y @ W1.T + b1.

Shapes (hardcoded): X,Y [8, 4, 1024, 64] f32; W1 [64,64]; b1,w2 [64]; b2 [].
Sharding: data-parallel over the fused B*H=32 head dim -> 4 heads per core
across 8 NeuronCores; W1/b1/w2/b2 replicated.

The kernel is HBM-bandwidth bound on the output stream, so the design
minimizes HBM bytes and keeps the output DMA queue dense:
- X/Y are cast to bf16 AND pre-transposed on the host into the exact
  d-on-partitions layout lin1 consumes (fully-contiguous input DMAs,
  half the input bytes, no on-device transpose/cast stage at all).
- the output is written as fp16 (half the output bytes) and upcast to
  f32 on the host after the gather; rounding is ~2^-11 rms, far
  inside the error budget.
- heads are processed in pairs packed into the two 64-row halves of
  the 128-partition dim, so every K=64 matmul runs 2x concurrent on
  the PE via tile_position row groups.
- X.T columns are ordered i = 8p + m, so score block m lands in PSUM
  with partition p <-> row i = 8p + m: an m-pair output tile
  [128, 2048] fp16 is 4 KiB/partition contiguous in DRAM. Y.T columns
  are plain j-order.
- PSUM: 2 lin1 banks + 3 double-bank score tiles = 8 banks, giving a
  3-deep score pipeline so the PE never stalls on evacuations.
- evacuations (the co-bottleneck: fp32-PSUM reads run at 1x on both
  ACT and DVE): score blocks alternate ACT (head 0) / DVE (head 1);
  lin1 bias fusions split 3 ACT / 1 DVE. The first m-pair of pair 0
  is evacuated in [128, 512] quarters across both engines to cut the
  latency to the first output DMA.
- input loads are split across both HWDGE rings (Y + consts on sync,
  X on scalar) because each dma_start occupies its issuing engine for
  ~0.6 us; output stores alternate between the sync HWDGE ring and
  the otherwise-idle GpSimd SWDGE ring so the store stream can reach
  the HBM write limit rather than a single ring's throughput.
- pair N+1's stage-1 chunks are threaded between pair N's score
  blocks. No PE warmup: stage-1 itself ramps the HAM clock gate, and
  at steady state the PE has ~2x headroom over the DMA pace even at
  half clock.
"""

import ml_dtypes
import numpy as np
from contextlib import ExitStack

import concourse.bass as bass
import concourse.tile as tile
from concourse import bacc, mybir
from concourse.bass_utils import run_bass_kernel_spmd

# If the caller's environment sets BASS_TRACE, run_bass_kernel_spmd's
# axon trace path imports antenv.axon_hooks, which not every image
# ships. Register a fallback so a stray BASS_TRACE can't crash the run
# (a None hook makes bass_utils skip tracing gracefully).
try:
    import antenv.axon_hooks  # noqa: F401
except ImportError:
    import sys
    import types

    _hooks = types.ModuleType("antenv.axon_hooks")
    _hooks._hook = None

    def _get_hook():
        return _hooks._hook

    def _set_hook(h):
        _hooks._hook = h

    _hooks.get_axon_ntff_profile_hook = _get_hook
    _hooks.set_axon_ntff_profile_hook = _set_hook
    sys.modules["antenv.axon_hooks"] = _hooks

B, H, L, D = 8, 4, 1024, 64
NCORES = 8
HPC = (B * H) // NCORES  # heads per core = 4
NPAIRS = HPC // 2

F32 = mybir.dt.float32
BF16 = mybir.dt.bfloat16
FP16 = mybir.dt.float16

LAST_RESULT = None
_CACHED_NC = None


def _build():
    nc = bacc.Bacc()
    # XT[P][s*64+d, 128m+p] = X[2P+s, 8p+m, d]
    XTd = nc.declare_dram_parameter("XT", [NPAIRS, 128, 1024], BF16, isOutput=False)
    # YT[P][s*64+d, j] = Y[2P+s, j, d]
    YTd = nc.declare_dram_parameter("YT", [NPAIRS, 128, 1024], BF16, isOutput=False)
    # CW = W1.T stacked 2x
    CWd = nc.declare_dram_parameter("CW", [128, 64], BF16, isOutput=False)
    Cd = nc.declare_dram_parameter("CONSTS", [128, 4], F32, isOutput=False)
    Od = nc.declare_dram_parameter("OUT", [HPC, L, L], FP16, isOutput=True)

    AF = mybir.ActivationFunctionType
    ALU = mybir.AluOpType

    with tile.TileContext(nc) as tc, ExitStack() as ctx:
        cpool = ctx.enter_context(tc.tile_pool(name="consts", bufs=1))
        xin_pool = ctx.enter_context(tc.tile_pool(name="xin", bufs=4))
        ab_pool = ctx.enter_context(tc.tile_pool(name="ab", bufs=4))
        out_pool = ctx.enter_context(tc.tile_pool(name="out", bufs=8))
        pf_pool = ctx.enter_context(tc.tile_pool(name="pf", bufs=2, space="PSUM"))
        ps_pool = ctx.enter_context(tc.tile_pool(name="ps", bufs=3, space="PSUM"))

        # sync ring: weights then Y pair-0 then the output stream.
        # scalar ring: consts then X pair-0 then the pair-1 tensors.
        # (each dma_start occupies its issuing engine ~0.6 us, so the
        # tiny const loads go first and the two big pair-0 tensors
        # dispatch concurrently on different rings.)
        xins = {}
        cw = cpool.tile([128, 64], BF16, tag="cw")
        nc.sync.dma_start(cw[:, :], CWd[:, :])
        consts = cpool.tile([128, 4], F32, tag="consts")
        nc.scalar.dma_start(consts[:, :], Cd[:, :])
        xins[(0, "b")] = xin_pool.tile([128, 1024], BF16, tag="xin", name="xinY0")
        nc.sync.dma_start(xins[(0, "b")][:, :], YTd[0, :, :])
        xins[(0, "a")] = xin_pool.tile([128, 1024], BF16, tag="xin", name="xinX0")
        nc.scalar.dma_start(xins[(0, "a")][:, :], XTd[0, :, :])
        for pr in range(1, NPAIRS):
            xins[(pr, "b")] = xin_pool.tile(
                [128, 1024], BF16, tag="xin", name=f"xinY{pr}"
            )
            nc.scalar.dma_start(xins[(pr, "b")][:, :], YTd[pr, :, :])
            xins[(pr, "a")] = xin_pool.tile(
                [128, 1024], BF16, tag="xin", name=f"xinX{pr}"
            )
            nc.scalar.dma_start(xins[(pr, "a")][:, :], XTd[pr, :, :])

        w1t2 = cw[:, 0:64]
        # Warm the PE past the HAM clock gate while the input loads are
        # in flight: the warm tile is memset on the otherwise-idle Pool
        # engine so the matmuls have no DMA dependency. Sized to end
        # about when pair-0 lands; stage-1 + scores then continue the
        # activity so the lift lands early in the score stream.
        warm_in = cpool.tile([128, 512], BF16, tag="warm")
        nc.gpsimd.memset(warm_in[:, :], 0.0)
        warm_ps = pf_pool.tile([128, 512], F32, tag="pf", name="warm_ps")
        for _ in range(4):
            nc.tensor.matmul(
                warm_ps[:, :],
                lhsT=warm_in[:, 0:128],
                rhs=warm_in[:, :],
                start=True,
                stop=True,
            )
        biasx = consts[:, 0:1]  # b1*w2 (stacked 2x)
        scalex = consts[:, 1:2]  # w2 (2x)
        biasy = consts[:, 2:3]  # b1 (2x)
        b2col = consts[:, 3:4]  # b2 broadcast

        def stage1_chunks(pair, ab):
            """Yield stage-1 (lin1) work as small closures so pair
            N+1's chain can be threaded between pair N's score
            blocks."""
            for nm in ("b", "a"):
                dst = ab_pool.tile([128, L], BF16, tag="ab", name=f"ab_{nm}")
                ab[nm] = dst

            def chunk_l(nm, n):
                xt = xins[(pair, nm)]
                dst = ab[nm]

                def run():
                    # lin1 for both heads concurrently on PE row groups
                    # 0-1 / 2-3; bias/scale fused on the PSUM->SBUF
                    # copy: A.T = (x + b1)*w2, B.T = y + b1. Y's n=1
                    # chunk rides DVE to balance engine load.
                    pf = pf_pool.tile([128, 512], F32, tag="pf")
                    for s in range(2):
                        rows = slice(64 * s, 64 * s + 64)
                        nc.tensor.matmul(
                            pf[rows, :],
                            lhsT=w1t2[rows, :],
                            rhs=xt[rows, bass.ts(n, 512)],
                            start=True,
                            stop=True,
                            tile_position=(64 * s, 64 * s),
                        )
                    if nm == "a" and n == 0:
                        # DVE, concurrent with b-n0 on ACT: the first
                        # score block needs both as early as possible.
                        nc.vector.tensor_scalar(
                            dst[:, bass.ts(n, 512)],
                            pf[:, :],
                            scalex,
                            biasx,
                            ALU.mult,
                            ALU.add,
                        )
                    elif nm == "a":
                        nc.scalar.activation(
                            dst[:, bass.ts(n, 512)],
                            pf[:, :],
                            AF.Identity,
                            bias=biasx,
                            scale=scalex,
                        )
                    else:
                        nc.scalar.activation(
                            dst[:, bass.ts(n, 512)],
                            pf[:, :],
                            AF.Identity,
                            bias=biasy,
                            scale=1.0,
                        )

                return run

            yield chunk_l("b", 0)
            yield chunk_l("a", 0)
            yield chunk_l("b", 1)
            yield chunk_l("a", 1)

        ab_cur = {}
        for ch in stage1_chunks(0, ab_cur):
            ch()
        dma_flip = 0
        for pair in range(NPAIRS):
            h0 = 2 * pair
            ab = ab_cur
            ab_next = {}
            next_chunks = (
                list(stage1_chunks(pair + 1, ab_next))
                if pair + 1 < NPAIRS
                else []
            )
            # scores: ps[p, j] = sum_d A.T[d, 8p+m] * B.T[d, j]; the two
            # heads of the pair run on disjoint PE row groups. Output
            # DMAs are grouped (1, 2, 2, 2, 1) m-blocks so the stream
            # starts one block earlier and the tail DMA is small, while
            # the interior keeps 4 KiB/partition descriptors.
            o = None
            group_of = {0: (0,), 1: (1, 2), 3: (3, 4), 5: (5, 6), 7: (7,)}
            for m in range(8):
                if next_chunks and 4 <= m < 4 + len(next_chunks):
                    next_chunks[m - 4]()
                if m in group_of:
                    grp = group_of[m]
                    o = [
                        out_pool.tile(
                            [128, 1024 * len(grp)],
                            FP16,
                            tag=f"o{s}",
                            name=f"o{s}",
                        )
                        for s in range(2)
                    ]
                gi = m - grp[0]
                for s in range(2):
                    rows = slice(64 * s, 64 * s + 64)
                    ps = ps_pool.tile([128, L], F32, tag="ps")
                    for n in range(2):
                        nc.tensor.matmul(
                            ps[:, bass.ts(n, 512)],
                            lhsT=ab["a"][rows, bass.ts(m, 128)],
                            rhs=ab["b"][rows, bass.ts(n, 512)],
                            start=True,
                            stop=True,
                            tile_position=(64 * s, 0),
                        )
                    dst = o[s][:, bass.ts(gi, 1024)]
                    if s == 0:
                        nc.scalar.activation(
                            dst, ps[:, :], AF.Relu, bias=b2col, scale=1.0
                        )
                    else:
                        nc.vector.tensor_scalar(
                            dst,
                            ps[:, :],
                            b2col,
                            0.0,
                            ALU.add,
                            ALU.max,
                        )
                if m == grp[-1]:
                    for s in range(2):
                        eng = nc.sync if dma_flip % 2 == 0 else nc.gpsimd
                        dma_flip += 1
                        eng.dma_start(
                            Od[h0 + s, :, :].rearrange(
                                "(p r) j -> p r j", r=8
                            )[:, grp[0] : grp[-1] + 1, :],
                            o[s][:, :].rearrange(
                                "p (mm j) -> p mm j", mm=len(grp)
                            ),
                        )
                if m < 4:
                    # keep-warm filler: a garbage matmul into the spare
                    # pf slot keeps PE duty high enough that the HAM
                    # clock gate holds K=8 through the score stream.
                    warm = pf_pool.tile([128, 512], F32, tag="pf", name="warm")
                    nc.tensor.matmul(
                        warm[0:64, :],
                        lhsT=w1t2[0:64, :],
                        rhs=xins[(pair, "b")][0:64, 0:512],
                        start=True,
                        stop=True,
                    )
            ab_cur = ab_next
    nc.compile()
    return nc


def _prep_inputs(X, Y, W1, b1, w2, b2):
    X = np.ascontiguousarray(np.asarray(X), dtype=np.float32).reshape(B * H, L, D)
    Y = np.ascontiguousarray(np.asarray(Y), dtype=np.float32).reshape(B * H, L, D)
    W1 = np.asarray(W1, dtype=np.float32)
    b1 = np.asarray(b1, dtype=np.float32)
    w2 = np.asarray(w2, dtype=np.float32)
    b2v = float(np.asarray(b2))

    # XT[c][P][s*64+d, 128m+p] = X[4c + 2P + s, 8p + m, d]
    XT = np.ascontiguousarray(
        X.reshape(NCORES, NPAIRS, 2, 128, 8, D)
        .transpose(0, 1, 2, 5, 4, 3)
        .reshape(NCORES, NPAIRS, 128, 1024)
        .astype(ml_dtypes.bfloat16)
    )
    # YT[c][P][s*64+d, j] = Y[4c + 2P + s, j, d]
    YT = np.ascontiguousarray(
        Y.reshape(NCORES, NPAIRS, 2, L, D)
        .transpose(0, 1, 2, 4, 3)
        .reshape(NCORES, NPAIRS, 128, 1024)
        .astype(ml_dtypes.bfloat16)
    )
    CW = np.ascontiguousarray(
        np.vstack([W1.T, W1.T]).astype(ml_dtypes.bfloat16)
    )
    consts = np.ascontiguousarray(
        np.stack(
            [
                np.tile(b1 * w2, 2),
                np.tile(w2, 2),
                np.tile(b1, 2),
                np.full(128, b2v, np.float32),
            ],
            axis=1,
        ),
        dtype=np.float32,
    )
    return XT, YT, CW, consts


def kernel(X, Y, W1, b1, w2, b2):
    global LAST_RESULT, _CACHED_NC
    XT, YT, CW, consts = _prep_inputs(X, Y, W1, b1, w2, b2)

    if _CACHED_NC is None:
        _CACHED_NC = _build()
    nc = _CACHED_NC

    in_maps = [
        {"XT": XT[i], "YT": YT[i], "CW": CW, "CONSTS": consts}
        for i in range(NCORES)
    ]
    res = run_bass_kernel_spmd(nc, in_maps, list(range(NCORES)))
    LAST_RESULT = res
    out = np.concatenate([res.results[i]["OUT"] for i in range(NCORES)], axis=0)
    return out.astype(np.float32).reshape(B, H, L, L)


# revision 8
# speedup vs baseline: 1.0493x; 1.0493x over previous
"""Trainium2 Bass kernel for AttentionNet:
out[b,h,i,j] = relu(sum_d w2[d] * Xf[b,h,i,d] * Yf[b,h,j,d] + b2)
where Xf = X @ W1.T + b1, Yf = Y @ W1.T + b1.

Shapes (hardcoded): X,Y [8, 4, 1024, 64] f32; W1 [64,64]; b1,w2 [64]; b2 [].
Sharding: data-parallel over the fused B*H=32 head dim -> 4 heads per core
across 8 NeuronCores; W1/b1/w2/b2 replicated.

The kernel is HBM-bandwidth bound on the output stream, so the design
minimizes HBM bytes and keeps the output DMA queue dense:
- X/Y are cast to bf16 AND pre-transposed on the host into the exact
  d-on-partitions layout lin1 consumes (fully-contiguous input DMAs,
  half the input bytes, no on-device transpose/cast stage at all).
- the output is written as fp16 (half the output bytes) and upcast to
  f32 on the host after the gather; rounding is ~2^-11 rms, far
  inside the error budget.
- heads are processed in pairs packed into the two 64-row halves of
  the 128-partition dim, so every K=64 matmul runs 2x concurrent on
  the PE via tile_position row groups.
- X.T columns are ordered i = 8p + m, so score block m lands in PSUM
  with partition p <-> row i = 8p + m: an m-pair output tile
  [128, 2048] fp16 is 4 KiB/partition contiguous in DRAM. Y.T columns
  are plain j-order.
- PSUM: 2 lin1 banks + 3 double-bank score tiles = 8 banks, giving a
  3-deep score pipeline so the PE never stalls on evacuations.
- evacuations (the co-bottleneck: fp32-PSUM reads run at 1x on both
  ACT and DVE): score blocks alternate ACT (head 0) / DVE (head 1);
  lin1 bias fusions split 3 ACT / 1 DVE. The first m-pair of pair 0
  is evacuated in [128, 512] quarters across both engines to cut the
  latency to the first output DMA.
- input loads are split across both HWDGE rings (Y + consts on sync,
  X on scalar) because each dma_start occupies its issuing engine for
  ~0.6 us; output stores alternate between the sync HWDGE ring and
  the otherwise-idle GpSimd SWDGE ring so the store stream can reach
  the HBM write limit rather than a single ring's throughput.
- pair N+1's stage-1 chunks are threaded between pair N's score
  blocks. No PE warmup: stage-1 itself ramps the HAM clock gate, and
  at steady state the PE has ~2x headroom over the DMA pace even at
  half clock.
"""

import ml_dtypes
import numpy as np
from contextlib import ExitStack

import concourse.bass as bass
import concourse.tile as tile
from concourse import bacc, mybir
from concourse.bass_utils import run_bass_kernel_spmd

# If the caller's environment sets BASS_TRACE, run_bass_kernel_spmd's
# axon trace path imports antenv.axon_hooks, which not every image
# ships. Register a fallback so a stray BASS_TRACE can't crash the run
# (a None hook makes bass_utils skip tracing gracefully).
try:
    import antenv.axon_hooks  # noqa: F401
except ImportError:
    import sys
    import types

    _hooks = types.ModuleType("antenv.axon_hooks")
    _hooks._hook = None

    def _get_hook():
        return _hooks._hook

    def _set_hook(h):
        _hooks._hook = h

    _hooks.get_axon_ntff_profile_hook = _get_hook
    _hooks.set_axon_ntff_profile_hook = _set_hook
    sys.modules["antenv.axon_hooks"] = _hooks

B, H, L, D = 8, 4, 1024, 64
NCORES = 8
HPC = (B * H) // NCORES  # heads per core = 4
NPAIRS = HPC // 2

F32 = mybir.dt.float32
BF16 = mybir.dt.bfloat16
FP16 = mybir.dt.float16

LAST_RESULT = None
_CACHED_NC = None


def _build():
    nc = bacc.Bacc()
    # XT[P][s*64+d, 128m+p] = X[2P+s, 8p+m, d]
    XTd = nc.declare_dram_parameter("XT", [NPAIRS, 128, 1024], BF16, isOutput=False)
    # YT[P][s*64+d, j] = Y[2P+s, j, d]
    YTd = nc.declare_dram_parameter("YT", [NPAIRS, 128, 1024], BF16, isOutput=False)
    # CW = W1.T stacked 2x
    CWd = nc.declare_dram_parameter("CW", [128, 64], BF16, isOutput=False)
    Cd = nc.declare_dram_parameter("CONSTS", [128, 4], F32, isOutput=False)
    Od = nc.declare_dram_parameter("OUT", [HPC, L, L], FP16, isOutput=True)

    AF = mybir.ActivationFunctionType
    ALU = mybir.AluOpType

    with tile.TileContext(nc) as tc, ExitStack() as ctx:
        cpool = ctx.enter_context(tc.tile_pool(name="consts", bufs=1))
        xin_pool = ctx.enter_context(tc.tile_pool(name="xin", bufs=4))
        ab_pool = ctx.enter_context(tc.tile_pool(name="ab", bufs=4))
        out_pool = ctx.enter_context(tc.tile_pool(name="out", bufs=8))
        pf_pool = ctx.enter_context(tc.tile_pool(name="pf", bufs=2, space="PSUM"))
        ps_pool = ctx.enter_context(tc.tile_pool(name="ps", bufs=3, space="PSUM"))

        # sync ring: weights then Y pair-0 then the output stream.
        # scalar ring: consts then X pair-0 then the pair-1 tensors.
        # (each dma_start occupies its issuing engine ~0.6 us, so the
        # tiny const loads go first and the two big pair-0 tensors
        # dispatch concurrently on different rings.)
        xins = {}
        cw = cpool.tile([128, 64], BF16, tag="cw")
        nc.sync.dma_start(cw[:, :], CWd[:, :])
        consts = cpool.tile([128, 4], F32, tag="consts")
        nc.scalar.dma_start(consts[:, :], Cd[:, :])
        xins[(0, "b")] = xin_pool.tile([128, 1024], BF16, tag="xin", name="xinY0")
        nc.sync.dma_start(xins[(0, "b")][:, :], YTd[0, :, :])
        xins[(0, "a")] = xin_pool.tile([128, 1024], BF16, tag="xin", name="xinX0")
        nc.scalar.dma_start(xins[(0, "a")][:, :], XTd[0, :, :])
        for pr in range(1, NPAIRS):
            xins[(pr, "b")] = xin_pool.tile(
                [128, 1024], BF16, tag="xin", name=f"xinY{pr}"
            )
            nc.scalar.dma_start(xins[(pr, "b")][:, :], YTd[pr, :, :])
            xins[(pr, "a")] = xin_pool.tile(
                [128, 1024], BF16, tag="xin", name=f"xinX{pr}"
            )
            nc.scalar.dma_start(xins[(pr, "a")][:, :], XTd[pr, :, :])

        w1t2 = cw[:, 0:64]
        # Warm the PE past the HAM clock gate while the input loads are
        # in flight: the warm tile is memset on the otherwise-idle Pool
        # engine so the matmuls have no DMA dependency. Sized to end
        # about when pair-0 lands; stage-1 + scores then continue the
        # activity so the lift lands early in the score stream.
        warm_in = cpool.tile([128, 512], BF16, tag="warm")
        nc.gpsimd.memset(warm_in[:, :], 0.0)
        # The Pool engine's preamble is short, so the memset (and thus
        # the warmup) starts ~4 us in -- well before the inputs land.
        # ~6 back-to-back N=512 matmuls (same PSUM tile: no pool
        # rotation waits) is the ~3.4 us of dense activity the HAM
        # needs to lift K=4 -> K=8; the rest run fast and keep the
        # window saturated until stage-1 takes over.
        warm_ps = pf_pool.tile([128, 512], F32, tag="pf", name="warm_ps")
        for _ in range(10):
            nc.tensor.matmul(
                warm_ps[:, :],
                lhsT=warm_in[:, 0:128],
                rhs=warm_in[:, :],
                start=True,
                stop=True,
            )
        biasx = consts[:, 0:1]  # b1*w2 (stacked 2x)
        scalex = consts[:, 1:2]  # w2 (2x)
        biasy = consts[:, 2:3]  # b1 (2x)
        b2col = consts[:, 3:4]  # b2 broadcast

        def stage1_chunks(pair, ab):
            """Yield stage-1 (lin1) work as small closures so pair
            N+1's chain can be threaded between pair N's score
            blocks."""
            for nm in ("b", "a"):
                dst = ab_pool.tile([128, L], BF16, tag="ab", name=f"ab_{nm}")
                ab[nm] = dst

            def chunk_l(nm, n):
                xt = xins[(pair, nm)]
                dst = ab[nm]

                def run():
                    # lin1 for both heads concurrently on PE row groups
                    # 0-1 / 2-3; bias/scale fused on the PSUM->SBUF
                    # copy: A.T = (x + b1)*w2, B.T = y + b1. Y's n=1
                    # chunk rides DVE to balance engine load.
                    pf = pf_pool.tile([128, 512], F32, tag="pf")
                    for s in range(2):
                        rows = slice(64 * s, 64 * s + 64)
                        nc.tensor.matmul(
                            pf[rows, :],
                            lhsT=w1t2[rows, :],
                            rhs=xt[rows, bass.ts(n, 512)],
                            start=True,
                            stop=True,
                            tile_position=(64 * s, 64 * s),
                        )
                    if nm == "a" and n == 0:
                        # DVE, concurrent with b-n0 on ACT: the first
                        # score block needs both as early as possible.
                        nc.vector.tensor_scalar(
                            dst[:, bass.ts(n, 512)],
                            pf[:, :],
                            scalex,
                            biasx,
                            ALU.mult,
                            ALU.add,
                        )
                    elif nm == "a":
                        nc.scalar.activation(
                            dst[:, bass.ts(n, 512)],
                            pf[:, :],
                            AF.Identity,
                            bias=biasx,
                            scale=scalex,
                        )
                    else:
                        nc.scalar.activation(
                            dst[:, bass.ts(n, 512)],
                            pf[:, :],
                            AF.Identity,
                            bias=biasy,
                            scale=1.0,
                        )

                return run

            yield chunk_l("b", 0)
            yield chunk_l("a", 0)
            yield chunk_l("b", 1)
            yield chunk_l("a", 1)

        ab_cur = {}
        for ch in stage1_chunks(0, ab_cur):
            ch()
        dma_flip = 0
        for pair in range(NPAIRS):
            h0 = 2 * pair
            ab = ab_cur
            ab_next = {}
            next_chunks = (
                list(stage1_chunks(pair + 1, ab_next))
                if pair + 1 < NPAIRS
                else []
            )
            # scores: ps[p, j] = sum_d A.T[d, 8p+m] * B.T[d, j]; the two
            # heads of the pair run on disjoint PE row groups. Output
            # DMAs are grouped (1, 2, 2, 2, 1) m-blocks so the stream
            # starts one block earlier and the tail DMA is small, while
            # the interior keeps 4 KiB/partition descriptors.
            o = None
            group_of = {0: (0,), 1: (1, 2), 3: (3, 4), 5: (5, 6), 7: (7,)}
            for m in range(8):
                if next_chunks and 4 <= m < 4 + len(next_chunks):
                    next_chunks[m - 4]()
                if m in group_of:
                    grp = group_of[m]
                    o = [
                        out_pool.tile(
                            [128, 1024 * len(grp)],
                            FP16,
                            tag=f"o{s}",
                            name=f"o{s}",
                        )
                        for s in range(2)
                    ]
                gi = m - grp[0]
                for s in range(2):
                    rows = slice(64 * s, 64 * s + 64)
                    ps = ps_pool.tile([128, L], F32, tag="ps")
                    for n in range(2):
                        nc.tensor.matmul(
                            ps[:, bass.ts(n, 512)],
                            lhsT=ab["a"][rows, bass.ts(m, 128)],
                            rhs=ab["b"][rows, bass.ts(n, 512)],
                            start=True,
                            stop=True,
                            tile_position=(64 * s, 0),
                        )
                    dst = o[s][:, bass.ts(gi, 1024)]
                    if s == 0:
                        nc.scalar.activation(
                            dst, ps[:, :], AF.Relu, bias=b2col, scale=1.0
                        )
                    else:
                        nc.vector.tensor_scalar(
                            dst,
                            ps[:, :],
                            b2col,
                            0.0,
                            ALU.add,
                            ALU.max,
                        )
                if m == grp[-1]:
                    for s in range(2):
                        eng = nc.sync if dma_flip % 2 == 0 else nc.gpsimd
                        dma_flip += 1
                        eng.dma_start(
                            Od[h0 + s, :, :].rearrange(
                                "(p r) j -> p r j", r=8
                            )[:, grp[0] : grp[-1] + 1, :],
                            o[s][:, :].rearrange(
                                "p (mm j) -> p mm j", mm=len(grp)
                            ),
                        )
                if m % 2 == 0:
                    # keep-warm filler: a garbage matmul into the spare
                    # pf slot keeps PE duty high enough that the HAM
                    # clock gate holds K=8 through the score stream.
                    warm = pf_pool.tile([128, 512], F32, tag="pf", name="warm")
                    nc.tensor.matmul(
                        warm[0:64, :],
                        lhsT=w1t2[0:64, :],
                        rhs=xins[(pair, "b")][0:64, 0:512],
                        start=True,
                        stop=True,
                    )
            ab_cur = ab_next
    nc.compile()
    return nc


def _prep_inputs(X, Y, W1, b1, w2, b2):
    X = np.ascontiguousarray(np.asarray(X), dtype=np.float32).reshape(B * H, L, D)
    Y = np.ascontiguousarray(np.asarray(Y), dtype=np.float32).reshape(B * H, L, D)
    W1 = np.asarray(W1, dtype=np.float32)
    b1 = np.asarray(b1, dtype=np.float32)
    w2 = np.asarray(w2, dtype=np.float32)
    b2v = float(np.asarray(b2))

    # XT[c][P][s*64+d, 128m+p] = X[4c + 2P + s, 8p + m, d]
    XT = np.ascontiguousarray(
        X.reshape(NCORES, NPAIRS, 2, 128, 8, D)
        .transpose(0, 1, 2, 5, 4, 3)
        .reshape(NCORES, NPAIRS, 128, 1024)
        .astype(ml_dtypes.bfloat16)
    )
    # YT[c][P][s*64+d, j] = Y[4c + 2P + s, j, d]
    YT = np.ascontiguousarray(
        Y.reshape(NCORES, NPAIRS, 2, L, D)
        .transpose(0, 1, 2, 4, 3)
        .reshape(NCORES, NPAIRS, 128, 1024)
        .astype(ml_dtypes.bfloat16)
    )
    CW = np.ascontiguousarray(
        np.vstack([W1.T, W1.T]).astype(ml_dtypes.bfloat16)
    )
    consts = np.ascontiguousarray(
        np.stack(
            [
                np.tile(b1 * w2, 2),
                np.tile(w2, 2),
                np.tile(b1, 2),
                np.full(128, b2v, np.float32),
            ],
            axis=1,
        ),
        dtype=np.float32,
    )
    return XT, YT, CW, consts


def kernel(X, Y, W1, b1, w2, b2):
    global LAST_RESULT, _CACHED_NC
    XT, YT, CW, consts = _prep_inputs(X, Y, W1, b1, w2, b2)

    if _CACHED_NC is None:
        _CACHED_NC = _build()
    nc = _CACHED_NC

    in_maps = [
        {"XT": XT[i], "YT": YT[i], "CW": CW, "CONSTS": consts}
        for i in range(NCORES)
    ]
    res = run_bass_kernel_spmd(nc, in_maps, list(range(NCORES)))
    LAST_RESULT = res
    out = np.concatenate([res.results[i]["OUT"] for i in range(NCORES)], axis=0)
    return out.astype(np.float32).reshape(B, H, L, L)


# revision 9
# speedup vs baseline: 1.0501x; 1.0008x over previous
"""Trainium2 Bass kernel for AttentionNet:
out[b,h,i,j] = relu(sum_d w2[d] * Xf[b,h,i,d] * Yf[b,h,j,d] + b2)
where Xf = X @ W1.T + b1, Yf = Y @ W1.T + b1.

Shapes (hardcoded): X,Y [8, 4, 1024, 64] f32; W1 [64,64]; b1,w2 [64]; b2 [].
Sharding: data-parallel over the fused B*H=32 head dim -> 4 heads per core
across 8 NeuronCores; W1/b1/w2/b2 replicated.

The kernel is HBM-bandwidth bound on the output stream, so the design
minimizes HBM bytes and keeps the output DMA queue dense:
- X/Y are cast to bf16 AND pre-transposed on the host into the exact
  d-on-partitions layout lin1 consumes (fully-contiguous input DMAs,
  half the input bytes, no on-device transpose/cast stage at all).
- the output is written as fp16 (half the output bytes) and upcast to
  f32 on the host after the gather; rounding is ~2^-11 rms, far
  inside the error budget.
- heads are processed in pairs packed into the two 64-row halves of
  the 128-partition dim, so every K=64 matmul runs 2x concurrent on
  the PE via tile_position row groups.
- X.T columns are ordered i = 8p + m, so score block m lands in PSUM
  with partition p <-> row i = 8p + m: an m-pair output tile
  [128, 2048] fp16 is 4 KiB/partition contiguous in DRAM. Y.T columns
  are plain j-order.
- PSUM: 2 lin1 banks + 3 double-bank score tiles = 8 banks, giving a
  3-deep score pipeline so the PE never stalls on evacuations.
- evacuations (the co-bottleneck: fp32-PSUM reads run at 1x on both
  ACT and DVE): score blocks alternate ACT (head 0) / DVE (head 1);
  lin1 bias fusions split 3 ACT / 1 DVE. The first m-pair of pair 0
  is evacuated in [128, 512] quarters across both engines to cut the
  latency to the first output DMA.
- input loads are split across both HWDGE rings (Y + consts on sync,
  X on scalar) because each dma_start occupies its issuing engine for
  ~0.6 us; output stores alternate between the sync HWDGE ring and
  the otherwise-idle GpSimd SWDGE ring so the store stream can reach
  the HBM write limit rather than a single ring's throughput.
- pair N+1's stage-1 chunks are threaded between pair N's score
  blocks. No PE warmup: stage-1 itself ramps the HAM clock gate, and
  at steady state the PE has ~2x headroom over the DMA pace even at
  half clock.
"""

import ml_dtypes
import numpy as np
from contextlib import ExitStack

import concourse.bass as bass
import concourse.tile as tile
from concourse import bacc, mybir
from concourse.bass_utils import run_bass_kernel_spmd

# If the caller's environment sets BASS_TRACE, run_bass_kernel_spmd's
# axon trace path imports antenv.axon_hooks, which not every image
# ships. Register a fallback so a stray BASS_TRACE can't crash the run
# (a None hook makes bass_utils skip tracing gracefully).
try:
    import antenv.axon_hooks  # noqa: F401
except ImportError:
    import sys
    import types

    _hooks = types.ModuleType("antenv.axon_hooks")
    _hooks._hook = None

    def _get_hook():
        return _hooks._hook

    def _set_hook(h):
        _hooks._hook = h

    _hooks.get_axon_ntff_profile_hook = _get_hook
    _hooks.set_axon_ntff_profile_hook = _set_hook
    sys.modules["antenv.axon_hooks"] = _hooks

B, H, L, D = 8, 4, 1024, 64
NCORES = 8
HPC = (B * H) // NCORES  # heads per core = 4
NPAIRS = HPC // 2

F32 = mybir.dt.float32
BF16 = mybir.dt.bfloat16
FP16 = mybir.dt.float16

LAST_RESULT = None
_CACHED_NC = None


def _build():
    nc = bacc.Bacc()
    # XT[P][s*64+d, 128m+p] = X[2P+s, 8p+m, d]
    XTd = nc.declare_dram_parameter("XT", [NPAIRS, 128, 1024], BF16, isOutput=False)
    # YT[P][s*64+d, j] = Y[2P+s, j, d]
    YTd = nc.declare_dram_parameter("YT", [NPAIRS, 128, 1024], BF16, isOutput=False)
    # CW = W1.T stacked 2x
    CWd = nc.declare_dram_parameter("CW", [128, 64], BF16, isOutput=False)
    Cd = nc.declare_dram_parameter("CONSTS", [128, 4], F32, isOutput=False)
    Od = nc.declare_dram_parameter("OUT", [HPC, L, L], FP16, isOutput=True)

    AF = mybir.ActivationFunctionType
    ALU = mybir.AluOpType

    with tile.TileContext(nc) as tc, ExitStack() as ctx:
        cpool = ctx.enter_context(tc.tile_pool(name="consts", bufs=1))
        xin_pool = ctx.enter_context(tc.tile_pool(name="xin", bufs=4))
        ab_pool = ctx.enter_context(tc.tile_pool(name="ab", bufs=4))
        out_pool = ctx.enter_context(tc.tile_pool(name="out", bufs=8))
        pf_pool = ctx.enter_context(tc.tile_pool(name="pf", bufs=2, space="PSUM"))
        ps_pool = ctx.enter_context(tc.tile_pool(name="ps", bufs=3, space="PSUM"))

        # sync ring: weights then Y pair-0 then the output stream.
        # scalar ring: consts then X pair-0 then the pair-1 tensors.
        # (each dma_start occupies its issuing engine ~0.6 us, so the
        # tiny const loads go first and the two big pair-0 tensors
        # dispatch concurrently on different rings.)
        xins = {}
        cw = cpool.tile([128, 64], BF16, tag="cw")
        nc.sync.dma_start(cw[:, :], CWd[:, :])
        consts = cpool.tile([128, 4], F32, tag="consts")
        nc.scalar.dma_start(consts[:, :], Cd[:, :])
        xins[(0, "b")] = xin_pool.tile([128, 1024], BF16, tag="xin", name="xinY0")
        nc.sync.dma_start(xins[(0, "b")][:, :], YTd[0, :, :])
        xins[(0, "a")] = xin_pool.tile([128, 1024], BF16, tag="xin", name="xinX0")
        nc.scalar.dma_start(xins[(0, "a")][:, :], XTd[0, :, :])
        for pr in range(1, NPAIRS):
            xins[(pr, "b")] = xin_pool.tile(
                [128, 1024], BF16, tag="xin", name=f"xinY{pr}"
            )
            nc.scalar.dma_start(xins[(pr, "b")][:, :], YTd[pr, :, :])
            xins[(pr, "a")] = xin_pool.tile(
                [128, 1024], BF16, tag="xin", name=f"xinX{pr}"
            )
            nc.scalar.dma_start(xins[(pr, "a")][:, :], XTd[pr, :, :])

        w1t2 = cw[:, 0:64]
        # Warm the PE past the HAM clock gate while the input loads are
        # in flight: the warm tile is memset on the otherwise-idle Pool
        # engine so the matmuls have no DMA dependency. Sized to end
        # about when pair-0 lands; stage-1 + scores then continue the
        # activity so the lift lands early in the score stream.
        warm_in = cpool.tile([128, 512], BF16, tag="warm")
        nc.gpsimd.memset(warm_in[:, :], 0.0)
        # The Pool engine's preamble is short, so the memset (and thus
        # the warmup) starts ~4 us in -- well before the inputs land.
        # ~6 back-to-back N=512 matmuls (same PSUM tile: no pool
        # rotation waits) is the ~3.4 us of dense activity the HAM
        # needs to lift K=4 -> K=8; the rest run fast and keep the
        # window saturated until stage-1 takes over.
        warm_ps = pf_pool.tile([128, 512], F32, tag="pf", name="warm_ps")

        def filler():
            # no-dependency PE matmul: keeps the HAM activity window
            # saturated so the clock gate holds K=8. Always writes the
            # same PSUM tile (same-engine WAW: no rotation waits).
            nc.tensor.matmul(
                warm_ps[:, :],
                lhsT=warm_in[:, 0:128],
                rhs=warm_in[:, :],
                start=True,
                stop=True,
            )

        for _ in range(10):
            filler()
        biasx = consts[:, 0:1]  # b1*w2 (stacked 2x)
        scalex = consts[:, 1:2]  # w2 (2x)
        biasy = consts[:, 2:3]  # b1 (2x)
        b2col = consts[:, 3:4]  # b2 broadcast

        def stage1_chunks(pair, ab):
            """Yield stage-1 (lin1) work as small closures so pair
            N+1's chain can be threaded between pair N's score
            blocks."""
            for nm in ("b", "a"):
                dst = ab_pool.tile([128, L], BF16, tag="ab", name=f"ab_{nm}")
                ab[nm] = dst

            def chunk_l(nm, n):
                xt = xins[(pair, nm)]
                dst = ab[nm]

                def run():
                    # lin1 for both heads concurrently on PE row groups
                    # 0-1 / 2-3; bias/scale fused on the PSUM->SBUF
                    # copy: A.T = (x + b1)*w2, B.T = y + b1. Y's n=1
                    # chunk rides DVE to balance engine load.
                    pf = pf_pool.tile([128, 512], F32, tag="pf")
                    for s in range(2):
                        rows = slice(64 * s, 64 * s + 64)
                        nc.tensor.matmul(
                            pf[rows, :],
                            lhsT=w1t2[rows, :],
                            rhs=xt[rows, bass.ts(n, 512)],
                            start=True,
                            stop=True,
                            tile_position=(64 * s, 64 * s),
                        )
                    if nm == "a" and n == 0:
                        # DVE, concurrent with b-n0 on ACT: the first
                        # score block needs both as early as possible.
                        nc.vector.tensor_scalar(
                            dst[:, bass.ts(n, 512)],
                            pf[:, :],
                            scalex,
                            biasx,
                            ALU.mult,
                            ALU.add,
                        )
                    elif nm == "a":
                        nc.scalar.activation(
                            dst[:, bass.ts(n, 512)],
                            pf[:, :],
                            AF.Identity,
                            bias=biasx,
                            scale=scalex,
                        )
                    else:
                        nc.scalar.activation(
                            dst[:, bass.ts(n, 512)],
                            pf[:, :],
                            AF.Identity,
                            bias=biasy,
                            scale=1.0,
                        )
                    filler()

                return run

            yield chunk_l("b", 0)
            yield chunk_l("a", 0)
            yield chunk_l("b", 1)
            yield chunk_l("a", 1)

        ab_cur = {}
        for ch in stage1_chunks(0, ab_cur):
            ch()
        dma_flip = 0
        for pair in range(NPAIRS):
            h0 = 2 * pair
            ab = ab_cur
            ab_next = {}
            next_chunks = (
                list(stage1_chunks(pair + 1, ab_next))
                if pair + 1 < NPAIRS
                else []
            )
            # scores: ps[p, j] = sum_d A.T[d, 8p+m] * B.T[d, j]; the two
            # heads of the pair run on disjoint PE row groups. Output
            # DMAs are grouped (1, 2, 2, 2, 1) m-blocks so the stream
            # starts one block earlier and the tail DMA is small, while
            # the interior keeps 4 KiB/partition descriptors.
            o = None
            group_of = {0: (0,), 1: (1, 2), 3: (3, 4), 5: (5, 6), 7: (7,)}
            for m in range(8):
                if next_chunks and 4 <= m < 4 + len(next_chunks):
                    next_chunks[m - 4]()
                if m in group_of:
                    grp = group_of[m]
                    o = [
                        out_pool.tile(
                            [128, 1024 * len(grp)],
                            FP16,
                            tag=f"o{s}",
                            name=f"o{s}",
                        )
                        for s in range(2)
                    ]
                gi = m - grp[0]
                for s in range(2):
                    rows = slice(64 * s, 64 * s + 64)
                    ps = ps_pool.tile([128, L], F32, tag="ps")
                    for n in range(2):
                        nc.tensor.matmul(
                            ps[:, bass.ts(n, 512)],
                            lhsT=ab["a"][rows, bass.ts(m, 128)],
                            rhs=ab["b"][rows, bass.ts(n, 512)],
                            start=True,
                            stop=True,
                            tile_position=(64 * s, 0),
                        )
                    dst = o[s][:, bass.ts(gi, 1024)]
                    if pair == 0 and m < 3:
                        # latency-critical stream start: halve the
                        # evacuation latency by splitting each block
                        # across both engines (shallower PE-duty dip,
                        # earlier first DMA).
                        nc.scalar.activation(
                            dst[:, 0:512],
                            ps[:, 0:512],
                            AF.Relu,
                            bias=b2col,
                            scale=1.0,
                        )
                        nc.vector.tensor_scalar(
                            dst[:, 512:1024],
                            ps[:, 512:1024],
                            b2col,
                            0.0,
                            ALU.add,
                            ALU.max,
                        )
                    elif s == 0:
                        nc.scalar.activation(
                            dst, ps[:, :], AF.Relu, bias=b2col, scale=1.0
                        )
                    else:
                        nc.vector.tensor_scalar(
                            dst,
                            ps[:, :],
                            b2col,
                            0.0,
                            ALU.add,
                            ALU.max,
                        )
                if m == grp[-1]:
                    for s in range(2):
                        eng = nc.sync if dma_flip % 2 == 0 else nc.gpsimd
                        dma_flip += 1
                        eng.dma_start(
                            Od[h0 + s, :, :].rearrange(
                                "(p r) j -> p r j", r=8
                            )[:, grp[0] : grp[-1] + 1, :],
                            o[s][:, :].rearrange(
                                "p (mm j) -> p mm j", mm=len(grp)
                            ),
                        )
                # keep-warm fillers: without them the PE duty dip at
                # the evacuation-paced stream start trips the HAM MID
                # window and the clock drops to K=4 for the whole
                # stream (where the PE itself becomes the bottleneck).
                filler()
                if m < 4:
                    filler()
            ab_cur = ab_next
    nc.compile()
    return nc


def _prep_inputs(X, Y, W1, b1, w2, b2):
    X = np.ascontiguousarray(np.asarray(X), dtype=np.float32).reshape(B * H, L, D)
    Y = np.ascontiguousarray(np.asarray(Y), dtype=np.float32).reshape(B * H, L, D)
    W1 = np.asarray(W1, dtype=np.float32)
    b1 = np.asarray(b1, dtype=np.float32)
    w2 = np.asarray(w2, dtype=np.float32)
    b2v = float(np.asarray(b2))

    # XT[c][P][s*64+d, 128m+p] = X[4c + 2P + s, 8p + m, d]
    XT = np.ascontiguousarray(
        X.reshape(NCORES, NPAIRS, 2, 128, 8, D)
        .transpose(0, 1, 2, 5, 4, 3)
        .reshape(NCORES, NPAIRS, 128, 1024)
        .astype(ml_dtypes.bfloat16)
    )
    # YT[c][P][s*64+d, j] = Y[4c + 2P + s, j, d]
    YT = np.ascontiguousarray(
        Y.reshape(NCORES, NPAIRS, 2, L, D)
        .transpose(0, 1, 2, 4, 3)
        .reshape(NCORES, NPAIRS, 128, 1024)
        .astype(ml_dtypes.bfloat16)
    )
    CW = np.ascontiguousarray(
        np.vstack([W1.T, W1.T]).astype(ml_dtypes.bfloat16)
    )
    consts = np.ascontiguousarray(
        np.stack(
            [
                np.tile(b1 * w2, 2),
                np.tile(w2, 2),
                np.tile(b1, 2),
                np.full(128, b2v, np.float32),
            ],
            axis=1,
        ),
        dtype=np.float32,
    )
    return XT, YT, CW, consts


def kernel(X, Y, W1, b1, w2, b2):
    global LAST_RESULT, _CACHED_NC
    XT, YT, CW, consts = _prep_inputs(X, Y, W1, b1, w2, b2)

    if _CACHED_NC is None:
        _CACHED_NC = _build()
    nc = _CACHED_NC

    in_maps = [
        {"XT": XT[i], "YT": YT[i], "CW": CW, "CONSTS": consts}
        for i in range(NCORES)
    ]
    res = run_bass_kernel_spmd(nc, in_maps, list(range(NCORES)))
    LAST_RESULT = res
    out = np.concatenate([res.results[i]["OUT"] for i in range(NCORES)], axis=0)
    return out.astype(np.float32).reshape(B, H, L, L)


# revision 11
# speedup vs baseline: 1.1113x; 1.0582x over previous
"""Trainium2 Bass kernel for AttentionNet:
out[b,h,i,j] = relu(sum_d w2[d] * Xf[b,h,i,d] * Yf[b,h,j,d] + b2)
where Xf = X @ W1.T + b1, Yf = Y @ W1.T + b1.

Shapes (hardcoded): X,Y [8, 4, 1024, 64] f32; W1 [64,64]; b1,w2 [64]; b2 [].
Sharding: data-parallel over the fused B*H=32 head dim -> 4 heads per core
across 8 NeuronCores; W1/b1/w2/b2 replicated.

The kernel is HBM-bandwidth bound on the output stream, so the design
minimizes HBM bytes and keeps the output DMA queue dense:
- X/Y are cast to bf16 AND pre-transposed on the host into the exact
  d-on-partitions layout lin1 consumes (fully-contiguous input DMAs,
  half the input bytes, no on-device transpose/cast stage at all).
- the output is written as fp16 (half the output bytes) and upcast to
  f32 on the host after the gather; rounding is ~2^-11 rms, far
  inside the error budget.
- heads are processed in pairs packed into the two 64-row halves of
  the 128-partition dim, so every K=64 matmul runs 2x concurrent on
  the PE via tile_position row groups.
- X.T columns are ordered i = 8p + m, so score block m lands in PSUM
  with partition p <-> row i = 8p + m: an m-pair output tile
  [128, 2048] fp16 is 4 KiB/partition contiguous in DRAM. Y.T columns
  are plain j-order.
- PSUM: 2 lin1 banks + 3 double-bank score tiles = 8 banks, giving a
  3-deep score pipeline so the PE never stalls on evacuations.
- evacuations (the co-bottleneck: fp32-PSUM reads run at 1x on both
  ACT and DVE): score blocks alternate ACT (head 0) / DVE (head 1);
  lin1 bias fusions split 3 ACT / 1 DVE. The first m-pair of pair 0
  is evacuated in [128, 512] quarters across both engines to cut the
  latency to the first output DMA.
- input loads are split across both HWDGE rings (Y + consts on sync,
  X on scalar) because each dma_start occupies its issuing engine for
  ~0.6 us; output stores alternate between the sync HWDGE ring and
  the otherwise-idle GpSimd SWDGE ring so the store stream can reach
  the HBM write limit rather than a single ring's throughput.
- pair N+1's stage-1 chunks are threaded between pair N's score
  blocks. No PE warmup: stage-1 itself ramps the HAM clock gate, and
  at steady state the PE has ~2x headroom over the DMA pace even at
  half clock.
"""

import ml_dtypes
import numpy as np
from contextlib import ExitStack

import concourse.bass as bass
import concourse.tile as tile
from concourse import bacc, mybir
from concourse.bass_utils import run_bass_kernel_spmd

# If the caller's environment sets BASS_TRACE, run_bass_kernel_spmd's
# axon trace path imports antenv.axon_hooks, which not every image
# ships. Register a fallback so a stray BASS_TRACE can't crash the run
# (a None hook makes bass_utils skip tracing gracefully).
try:
    import antenv.axon_hooks  # noqa: F401
except ImportError:
    import sys
    import types

    _hooks = types.ModuleType("antenv.axon_hooks")
    _hooks._hook = None

    def _get_hook():
        return _hooks._hook

    def _set_hook(h):
        _hooks._hook = h

    _hooks.get_axon_ntff_profile_hook = _get_hook
    _hooks.set_axon_ntff_profile_hook = _set_hook
    sys.modules["antenv.axon_hooks"] = _hooks

B, H, L, D = 8, 4, 1024, 64
NCORES = 8
HPC = (B * H) // NCORES  # heads per core = 4
NPAIRS = HPC // 2

F32 = mybir.dt.float32
BF16 = mybir.dt.bfloat16
FP16 = mybir.dt.float16

LAST_RESULT = None
_CACHED_NC = None


def _build():
    nc = bacc.Bacc()
    # XT[P][s*64+d, 128m+p] = X[2P+s, 8p+m, d]
    XTd = nc.declare_dram_parameter("XT", [NPAIRS, 128, 1024], BF16, isOutput=False)
    # YT[P][s*64+d, j] = Y[2P+s, j, d]
    YTd = nc.declare_dram_parameter("YT", [NPAIRS, 128, 1024], BF16, isOutput=False)
    # CW = W1.T stacked 2x
    CWd = nc.declare_dram_parameter("CW", [128, 64], BF16, isOutput=False)
    Cd = nc.declare_dram_parameter("CONSTS", [128, 4], F32, isOutput=False)
    Od = nc.declare_dram_parameter("OUT", [HPC, L, L], FP16, isOutput=True)

    AF = mybir.ActivationFunctionType
    ALU = mybir.AluOpType

    with tile.TileContext(nc) as tc, ExitStack() as ctx:
        cpool = ctx.enter_context(tc.tile_pool(name="consts", bufs=1))
        xin_pool = ctx.enter_context(tc.tile_pool(name="xin", bufs=4))
        ab_pool = ctx.enter_context(tc.tile_pool(name="ab", bufs=4))
        out_pool = ctx.enter_context(tc.tile_pool(name="out", bufs=8))
        pf_pool = ctx.enter_context(tc.tile_pool(name="pf", bufs=2, space="PSUM"))
        ps_pool = ctx.enter_context(tc.tile_pool(name="ps", bufs=3, space="PSUM"))

        # sync ring: weights then Y pair-0 then the output stream.
        # scalar ring: consts then X pair-0 then the pair-1 tensors.
        # (each dma_start occupies its issuing engine ~0.6 us, so the
        # tiny const loads go first and the two big pair-0 tensors
        # dispatch concurrently on different rings.)
        xins = {}
        cw = cpool.tile([128, 64], BF16, tag="cw")
        nc.sync.dma_start(cw[:, :], CWd[:, :])
        consts = cpool.tile([128, 4], F32, tag="consts")
        nc.scalar.dma_start(consts[:, :], Cd[:, :])
        xins[(0, "b")] = xin_pool.tile([128, 1024], BF16, tag="xin", name="xinY0")
        nc.sync.dma_start(xins[(0, "b")][:, :], YTd[0, :, :])
        xins[(0, "a")] = xin_pool.tile([128, 1024], BF16, tag="xin", name="xinX0")
        nc.scalar.dma_start(xins[(0, "a")][:, :], XTd[0, :, :])
        for pr in range(1, NPAIRS):
            xins[(pr, "b")] = xin_pool.tile(
                [128, 1024], BF16, tag="xin", name=f"xinY{pr}"
            )
            nc.scalar.dma_start(xins[(pr, "b")][:, :], YTd[pr, :, :])
            xins[(pr, "a")] = xin_pool.tile(
                [128, 1024], BF16, tag="xin", name=f"xinX{pr}"
            )
            nc.scalar.dma_start(xins[(pr, "a")][:, :], XTd[pr, :, :])

        w1t2 = cw[:, 0:64]
        # Warm the PE past the HAM clock gate while the input loads are
        # in flight: the warm tile is memset on the otherwise-idle Pool
        # engine so the matmuls have no DMA dependency. Sized to end
        # about when pair-0 lands; stage-1 + scores then continue the
        # activity so the lift lands early in the score stream.
        warm_in = cpool.tile([128, 512], BF16, tag="warm")
        nc.gpsimd.memset(warm_in[:, :], 0.0)
        # The Pool engine's preamble is short, so the memset (and thus
        # the warmup) starts well before the inputs land. ~6
        # back-to-back N=512 matmuls into one never-read ps tile (no
        # pool-rotation waits) is the ~3.4 us of dense activity the
        # HAM needs to lift the PE clock gate from K=4 to K=8; the
        # tile's slot recycles into the score rotation afterwards.
        warm_ps = ps_pool.tile([128, L], F32, tag="ps", name="warm_ps")
        for _ in range(6):
            nc.tensor.matmul(
                warm_ps[:, 0:512],
                lhsT=warm_in[:, 0:128],
                rhs=warm_in[:, 0:512],
                start=True,
                stop=True,
            )
        biasx = consts[:, 0:1]  # b1*w2 (stacked 2x)
        scalex = consts[:, 1:2]  # w2 (2x)
        biasy = consts[:, 2:3]  # b1 (2x)
        b2col = consts[:, 3:4]  # b2 broadcast

        def stage1_chunks(pair, ab):
            """Yield stage-1 (lin1) work as 8 single-matmul sub-chunks
            so pair N+1's PE load spreads evenly across all 8 of pair
            N's score slots (at K=4 a full chunk per slot overruns the
            DMA pace)."""
            pfs = {}
            for nm in ("b", "a"):
                dst = ab_pool.tile([128, L], BF16, tag="ab", name=f"ab_{nm}")
                ab[nm] = dst

            def sub(nm, n, s):
                xt = xins[(pair, nm)]
                dst = ab[nm]

                def run():
                    # lin1; bias/scale fused on the PSUM->SBUF copy:
                    # A.T = x*w2 + b1*w2, B.T = y + b1. a-n0 rides DVE
                    # (concurrent with b's ACT evacs at stream start).
                    if s == 0:
                        pfs[nm] = pf_pool.tile(
                            [128, 512], F32, tag="pf", name="pf"
                        )
                    pf = pfs[nm]
                    rows = slice(64 * s, 64 * s + 64)
                    nc.tensor.matmul(
                        pf[rows, :],
                        lhsT=w1t2[rows, :],
                        rhs=xt[rows, bass.ts(n, 512)],
                        start=True,
                        stop=True,
                        tile_position=(64 * s, 64 * s),
                    )
                    if s == 0:
                        return
                    if nm == "a" and n == 0:
                        nc.vector.tensor_scalar(
                            dst[:, bass.ts(n, 512)],
                            pf[:, :],
                            scalex,
                            biasx,
                            ALU.mult,
                            ALU.add,
                        )
                    elif nm == "a":
                        nc.scalar.activation(
                            dst[:, bass.ts(n, 512)],
                            pf[:, :],
                            AF.Identity,
                            bias=biasx,
                            scale=scalex,
                        )
                    else:
                        nc.scalar.activation(
                            dst[:, bass.ts(n, 512)],
                            pf[:, :],
                            AF.Identity,
                            bias=biasy,
                            scale=1.0,
                        )

                return run

            for nm, n in (("b", 0), ("a", 0), ("b", 1), ("a", 1)):
                yield sub(nm, n, 0)
                yield sub(nm, n, 1)

        ab_cur = {}
        for ch in stage1_chunks(0, ab_cur):
            ch()
        dma_flip = 0
        for pair in range(NPAIRS):
            h0 = 2 * pair
            ab = ab_cur
            ab_next = {}
            next_chunks = (
                list(stage1_chunks(pair + 1, ab_next))
                if pair + 1 < NPAIRS
                else []
            )
            # scores: ps[p, j] = sum_d A.T[d, 8p+m] * B.T[d, j]; the two
            # heads of the pair run on disjoint PE row groups. Output
            # DMAs are grouped (1, 2, 2, 2, 1) m-blocks so the stream
            # starts one block earlier and the tail DMA is small, while
            # the interior keeps 4 KiB/partition descriptors.
            o = None
            group_of = {0: (0,), 1: (1, 2), 3: (3, 4), 5: (5, 6), 7: (7,)}
            for m in range(8):
                if next_chunks:
                    next_chunks[m]()
                if m in group_of:
                    grp = group_of[m]
                    o = [
                        out_pool.tile(
                            [128, 1024 * len(grp)],
                            FP16,
                            tag=f"o{s}",
                            name=f"o{s}",
                        )
                        for s in range(2)
                    ]
                gi = m - grp[0]
                for s in range(2):
                    rows = slice(64 * s, 64 * s + 64)
                    ps = ps_pool.tile([128, L], F32, tag="ps")
                    if s == 1:
                        # keep-warm filler: a redundant pre-write of
                        # this block (overwritten by the real matmul
                        # below). Without the extra PE activity the
                        # duty dip at the evacuation-paced stream pace
                        # trips the HAM MID window and the clock drops
                        # to K=4 for the rest of the stream.
                        nc.tensor.matmul(
                            ps[:, 0 : (512 if m < 4 else 256)],
                            lhsT=ab["a"][rows, bass.ts(m, 128)],
                            rhs=ab["b"][rows, 0 : (512 if m < 4 else 256)],
                            start=True,
                            stop=True,
                            tile_position=(64 * s, 0),
                        )
                    for n in range(2):
                        nc.tensor.matmul(
                            ps[:, bass.ts(n, 512)],
                            lhsT=ab["a"][rows, bass.ts(m, 128)],
                            rhs=ab["b"][rows, bass.ts(n, 512)],
                            start=True,
                            stop=True,
                            tile_position=(64 * s, 0),
                        )
                    dst = o[s][:, bass.ts(gi, 1024)]
                    if pair == 0 and m < 3:
                        # latency-critical stream start: halve the
                        # evacuation latency by splitting each block
                        # across both engines (shallower PE-duty dip,
                        # earlier first DMA).
                        nc.scalar.activation(
                            dst[:, 0:512],
                            ps[:, 0:512],
                            AF.Relu,
                            bias=b2col,
                            scale=1.0,
                        )
                        nc.vector.tensor_scalar(
                            dst[:, 512:1024],
                            ps[:, 512:1024],
                            b2col,
                            0.0,
                            ALU.add,
                            ALU.max,
                        )
                    elif s == 0:
                        nc.scalar.activation(
                            dst, ps[:, :], AF.Relu, bias=b2col, scale=1.0
                        )
                    else:
                        nc.vector.tensor_scalar(
                            dst,
                            ps[:, :],
                            b2col,
                            0.0,
                            ALU.add,
                            ALU.max,
                        )
                if m == grp[-1]:
                    for s in range(2):
                        eng = nc.sync if dma_flip % 2 == 0 else nc.gpsimd
                        dma_flip += 1
                        eng.dma_start(
                            Od[h0 + s, :, :].rearrange(
                                "(p r) j -> p r j", r=8
                            )[:, grp[0] : grp[-1] + 1, :],
                            o[s][:, :].rearrange(
                                "p (mm j) -> p mm j", mm=len(grp)
                            ),
                        )

            ab_cur = ab_next
    nc.compile()
    return nc


def _prep_inputs(X, Y, W1, b1, w2, b2):
    X = np.ascontiguousarray(np.asarray(X), dtype=np.float32).reshape(B * H, L, D)
    Y = np.ascontiguousarray(np.asarray(Y), dtype=np.float32).reshape(B * H, L, D)
    W1 = np.asarray(W1, dtype=np.float32)
    b1 = np.asarray(b1, dtype=np.float32)
    w2 = np.asarray(w2, dtype=np.float32)
    b2v = float(np.asarray(b2))

    # XT[c][P][s*64+d, 128m+p] = X[4c + 2P + s, 8p + m, d]
    XT = np.ascontiguousarray(
        X.reshape(NCORES, NPAIRS, 2, 128, 8, D)
        .transpose(0, 1, 2, 5, 4, 3)
        .reshape(NCORES, NPAIRS, 128, 1024)
        .astype(ml_dtypes.bfloat16)
    )
    # YT[c][P][s*64+d, j] = Y[4c + 2P + s, j, d]
    YT = np.ascontiguousarray(
        Y.reshape(NCORES, NPAIRS, 2, L, D)
        .transpose(0, 1, 2, 4, 3)
        .reshape(NCORES, NPAIRS, 128, 1024)
        .astype(ml_dtypes.bfloat16)
    )
    CW = np.ascontiguousarray(
        np.vstack([W1.T, W1.T]).astype(ml_dtypes.bfloat16)
    )
    consts = np.ascontiguousarray(
        np.stack(
            [
                np.tile(b1 * w2, 2),
                np.tile(w2, 2),
                np.tile(b1, 2),
                np.full(128, b2v, np.float32),
            ],
            axis=1,
        ),
        dtype=np.float32,
    )
    return XT, YT, CW, consts


def kernel(X, Y, W1, b1, w2, b2):
    global LAST_RESULT, _CACHED_NC
    XT, YT, CW, consts = _prep_inputs(X, Y, W1, b1, w2, b2)

    if _CACHED_NC is None:
        _CACHED_NC = _build()
    nc = _CACHED_NC

    in_maps = [
        {"XT": XT[i], "YT": YT[i], "CW": CW, "CONSTS": consts}
        for i in range(NCORES)
    ]
    res = run_bass_kernel_spmd(nc, in_maps, list(range(NCORES)))
    LAST_RESULT = res
    out = np.concatenate([res.results[i]["OUT"] for i in range(NCORES)], axis=0)
    return out.astype(np.float32).reshape(B, H, L, L)
